# revision 2
# baseline (speedup 1.0000x reference)
"""Self-contained Trainium2 Bass kernel: ChildSum TreeLSTM forest encoder.

Forest of B=4 full 4-ary trees, depth 8 (87381 nodes/tree), E=H=128.
Sharding: 8 cores, each owns half a tree (the 2 subtrees rooted at two of the
root's four children = 43690 nodes). Levels 0..2 run on-device with no
cross-core communication; levels 3..7 and the root are combined on host.

Device layout: transposed [H=128 partitions, nodes free], fp16 end-to-end
(matmuls fp16 -> f32 PSUM; elementwise fp16 on DVE at the 2x_1p rate). Host
pre-transposes x per core and permutes each level's nodes into child-major
order so every child gather on device is a unit-stride slice.

The emission is software-pipelined around the Activation engine (the binding
engine: 8 transcendental columns per internal node, 4 per leaf, 1 col/cycle):
leaf groups of 512 and 512-node level-chunks are staggered so every ACTIVATE's
inputs are ready before the engine reaches it.
"""

import numpy as np

try:
    import concourse.bass as bass
except ImportError:  # pragma: no cover - env fallback
    import sys

    for _p in (
        "/opt/trn_rl_repo",
        "/root/.axon_site/_ro/trn_rl_repo",
        "/root/.axon_site/_ro/pypackages",
        "/root/.axon_site",
    ):
        if _p not in sys.path:
            sys.path.append(_p)
    import concourse.bass as bass

from contextlib import ExitStack

import concourse.tile as tile
from concourse import mybir
from concourse.bass_utils import run_bass_kernel_spmd

# ---- problem geometry (hardcoded) ----
B, E, H, D, BR = 4, 128, 128, 8, 4
LEVEL_SIZES = [BR ** (D - l) for l in range(D + 1)]  # leaves ... root
OFFSETS = [0]
for _n in LEVEL_SIZES:
    OFFSETS.append(OFFSETS[-1] + _n)
N_NODES = OFFSETS[-1]  # 87381

NCORES = 8
NL = [2 * 4 ** (7 - l) for l in range(8)]  # per-core level sizes 32768..2
LOFF = [0]
for _n in NL:
    LOFF.append(LOFF[-1] + _n)
NCOLS = LOFF[-1]  # 43690

SCW = 512  # super-chunk width (nodes per internal unit / leaves per group)
CH = 256  # internal PSUM/act chunk

F32 = mybir.dt.float32
F16 = mybir.dt.float16
SIG = mybir.ActivationFunctionType.Sigmoid
TANH = mybir.ActivationFunctionType.Tanh


def _split_excess_waits(nc, limit=1):
    """Walrus codegen only accepts `limit` sem-waits per instruction; hoist
    extras into preceding same-engine NoOps."""
    ctr = 0
    for bb in nc.m.functions[0].blocks:
        new_insts = []
        for inst in bb.instructions:
            si = inst.sync_info
            if si is not None and si.on_wait and len(si.on_wait) > limit:
                waits = list(si.on_wait)
                extra, keep = waits[:-limit], waits[-limit:]
                for i in range(0, len(extra), limit):
                    ctr += 1
                    new_insts.append(
                        mybir.InstNoOp(
                            name=f"wait-split-{ctr}",
                            engine=inst.engine,
                            ins=[],
                            outs=[],
                            sync_info=mybir.SyncInfo(
                                on_wait=extra[i : i + limit], on_update=[]
                            ),
                        )
                    )
                inst.sync_info = mybir.SyncInfo(
                    on_wait=keep, on_update=list(si.on_update or [])
                )
            new_insts.append(inst)
        bb.instructions[:] = new_insts
    return ctr


def _build_program(zero_bias: bool, repeats: int = 1):
    nc = bass.Bass("TRN2", target_bir_lowering=False, debug=False)
    xt_d = nc.dram_tensor("xt", [128, NCOLS], F16, kind="ExternalInput")
    wx_d = nc.dram_tensor("wx", [128, 512], F16, kind="ExternalInput")
    uiou_d = nc.dram_tensor("uiou", [128, 384], F16, kind="ExternalInput")
    uf_d = nc.dram_tensor("uf", [128, 128], F16, kind="ExternalInput")
    b_d = nc.dram_tensor("bias", [128, 4], F32, kind="ExternalInput")
    out_d = nc.dram_tensor("out", [128, 4096], F16, kind="ExternalOutput")

    with tile.TileContext(nc) as tc, ExitStack() as es:
        wp = es.enter_context(tc.tile_pool(name="w", bufs=1))
        store = es.enter_context(tc.tile_pool(name="store", bufs=1))
        leafp = es.enter_context(tc.tile_pool(name="leafsc", bufs=2))
        xp = es.enter_context(tc.tile_pool(name="x", bufs=2))
        lg = es.enter_context(tc.tile_pool(name="lg", bufs=3))
        ig = es.enter_context(tc.tile_pool(name="ig", bufs=3))
        mw = es.enter_context(tc.tile_pool(name="mw", bufs=2))
        pp = es.enter_context(tc.tile_pool(name="ps", bufs=2, space="PSUM"))

        # weights
        wx = wp.tile([128, 512], F16, tag="wx")
        uiou = wp.tile([128, 384], F16, tag="uiou")
        uf = wp.tile([128, 128], F16, tag="uf")
        bias = wp.tile([128, 4], F32, tag="bias")
        warm = wp.tile([128, 1], F32, tag="warm")
        nc.vector.memset(warm[:], 0.0)
        nc.scalar.activation(warm[:], warm[:], SIG)
        nc.scalar.activation(warm[:], warm[:], TANH)
        nc.sync.dma_start(wx[:], wx_d.ap())
        nc.sync.dma_start(uiou[:], uiou_d.ap())
        nc.sync.dma_start(uf[:], uf_d.ap())
        nc.sync.dma_start(bias[:], b_d.ap())
        b_i, b_f, b_o, b_u = (bias[:, g : g + 1] for g in range(4))

        WXI, WXF, WXO, WXU = (wx[:, g * 128 : (g + 1) * 128] for g in range(4))
        UI, UO, UU = (uiou[:, g * 128 : (g + 1) * 128] for g in range(3))

        # persistent per-level stores (levels 1..2): fp16
        h_st = {
            1: store.tile([128, NL[1]], F16, tag="h1", name="h_st1"),
            2: store.tile([128, NL[2]], F16, tag="h2", name="h_st2"),
        }
        c_st = {
            1: store.tile([128, NL[1]], F16, tag="c1", name="c_st1"),
            2: store.tile([128, NL[2]], F16, tag="c2", name="c_st2"),
        }

        xt_leaf3d = xt_d.ap()[:, 0 : 4 * NL[1]].rearrange("p (k c) -> p k c", k=4)

        def leaf_phase(sc):
            """512-leaf groups j=0..3 (child-blocks) for L1 cols
            [sc*512,(sc+1)*512). Emits pre0,pre1,post0,pre2,post1,pre3,
            post2,post3 so every ACTIVATE's input is ready in time."""
            xleaf = xp.tile([128, 4, SCW], F16, tag="xleaf", name="xleaf")
            nc.sync.dma_start(
                xleaf[:], xt_leaf3d[:, :, sc * SCW : (sc + 1) * SCW]
            )
            h0 = leafp.tile([128, 4, SCW], F16, tag="h0", name="h0")
            c0 = leafp.tile([128, 4, SCW], F16, tag="c0", name="c0")
            posts = []

            def pre(j):
                giop = lg.tile([128, 1024], F16, tag="gio0", name="gio0")
                gu = lg.tile([128, SCW], F16, tag="gu0", name="gu0")
                ps_io = pp.tile([128, 1024], F32, tag="psio", name="psio")
                ps_u = pp.tile([128, SCW], F32, tag="psu", bufs=4, name="psu")
                xg = xleaf[:, j, :]
                nc.tensor.matmul(ps_io[:, 0:512], WXI, xg, start=True, stop=True)
                nc.tensor.matmul(ps_io[:, 512:1024], WXO, xg, start=True, stop=True)
                nc.tensor.matmul(ps_u[:], WXU, xg, start=True, stop=True)
                if zero_bias:
                    nc.scalar.activation(giop[:], ps_io[:], SIG)
                else:
                    nc.scalar.activation(giop[:, 0:512], ps_io[:, 0:512], SIG, bias=b_i)
                    nc.scalar.activation(
                        giop[:, 512:1024], ps_io[:, 512:1024], SIG, bias=b_o
                    )
                nc.scalar.activation(gu[:], ps_u[:], TANH, bias=b_u)
                nc.vector.tensor_mul(c0[:, j, :], giop[:, 0:512], gu[:])

                def post():
                    tct = lg.tile([128, SCW], F16, tag="tct0", name="tct0")
                    nc.scalar.activation(tct[:], c0[:, j, :], TANH)
                    nc.gpsimd.tensor_mul(h0[:, j, :], giop[:, 512:1024], tct[:])

                posts.append(post)

            pre(0)
            pre(1)
            posts[0]()
            pre(2)
            posts[1]()
            pre(3)
            posts[2]()
            posts[3]()
            return h0, c0

        def internal_phase(l, u, hch, cch):
            """One 512-node unit of level l at cols [u*512,(u+1)*512).

            hch(k, a, b) / cch(k, a, b): AP of child-k cols [a,b) within the
            unit (fp16). Emits hs-sum, two 256 PSUM/act chunks, and the
            SC-wide elementwise chain; returns the deferred tail
            (tanh(c), h mul) to emit ~2 slots later."""
            q0 = u * SCW
            gio = ig.tile([128, 1024], F16, tag="gio1", name="gio1")
            gf = ig.tile([128, 2048], F16, tag="gf1", name="gf1")
            gu = ig.tile([128, SCW], F16, tag="gu1", bufs=2, name="gu1")
            xv = xp.tile([128, SCW], F16, tag="xint", name="xint")
            nc.sync.dma_start(xv[:], xt_d.ap()[:, LOFF[l] + q0 : LOFF[l] + q0 + SCW])

            t0 = mw.tile([128, SCW], F16, tag="t0", name="t0")
            t1 = mw.tile([128, SCW], F16, tag="t1", name="t1")
            hsv = mw.tile([128, SCW], F16, tag="hsv", name="hsv")
            nc.vector.tensor_add(t0[:], hch(0, 0, SCW), hch(1, 0, SCW))
            nc.gpsimd.tensor_add(t1[:], hch(2, 0, SCW), hch(3, 0, SCW))
            nc.vector.tensor_add(hsv[:], t0[:], t1[:])

            gio_v = gio[:].rearrange("p (b a c) -> p b a c", b=2, a=2)
            gf_v = gf[:].rearrange("p (f a c) -> p f a c", f=4, a=2)
            for k in range(2):
                q = k * CH
                ps_io = pp.tile([128, 512], F32, tag="psu", bufs=4, name="psu")
                nc.tensor.matmul(
                    ps_io[:, 0:CH], WXI, xv[:, q : q + CH], start=True, stop=False
                )
                nc.tensor.matmul(
                    ps_io[:, 0:CH], UI, hsv[:, q : q + CH], start=False, stop=True
                )
                nc.tensor.matmul(
                    ps_io[:, CH:512], WXO, xv[:, q : q + CH], start=True, stop=False
                )
                nc.tensor.matmul(
                    ps_io[:, CH:512], UO, hsv[:, q : q + CH], start=False, stop=True
                )
                ps_f = pp.tile([128, 1024], F32, tag="psio", name="psf")
                for ch in range(4):
                    s = ch * CH
                    nc.tensor.matmul(
                        ps_f[:, s : s + CH], WXF, xv[:, q : q + CH], start=True, stop=False
                    )
                    nc.tensor.matmul(
                        ps_f[:, s : s + CH], uf[:], hch(ch, q, q + CH), start=False, stop=True
                    )
                ps_u = pp.tile([128, 512], F32, tag="psu", bufs=4, name="psu2")
                nc.tensor.matmul(
                    ps_u[:, 0:CH], WXU, xv[:, q : q + CH], start=True, stop=False
                )
                nc.tensor.matmul(
                    ps_u[:, 0:CH], UU, hsv[:, q : q + CH], start=False, stop=True
                )

                if zero_bias:
                    nc.scalar.activation(
                        gio_v[:, :, k, :],
                        ps_io[:].rearrange("p (b c) -> p b c", b=2),
                        SIG,
                    )
                else:
                    nc.scalar.activation(
                        gio_v[:, 0, k, :], ps_io[:, 0:CH], SIG, bias=b_i
                    )
                    nc.scalar.activation(
                        gio_v[:, 1, k, :], ps_io[:, CH:512], SIG, bias=b_o
                    )
                nc.scalar.activation(
                    gf_v[:, :, k, :],
                    ps_f[:].rearrange("p (f c) -> p f c", f=4),
                    SIG,
                    bias=b_f,
                )
                nc.scalar.activation(
                    gu[:, q : q + CH], ps_u[:, 0:CH], TANH, bias=b_u
                )

            # SC-wide elementwise: c = i*u + sum_k f_k*c_k
            m0 = mw.tile([128, SCW], F16, tag="m0", name="m0")
            m1 = mw.tile([128, SCW], F16, tag="m1", name="m1")
            m2 = mw.tile([128, SCW], F16, tag="m2", name="m2")
            m3 = mw.tile([128, SCW], F16, tag="m3", name="m3")
            s01 = mw.tile([128, SCW], F16, tag="s01", name="s01")
            s23 = mw.tile([128, SCW], F16, tag="s23", name="s23")
            fc = mw.tile([128, SCW], F16, tag="fc", name="fc")
            iu = mw.tile([128, SCW], F16, tag="iu", name="iu")
            nc.vector.tensor_mul(m0[:], gf[:, 0:512], cch(0, 0, SCW))
            nc.vector.tensor_mul(m1[:], gf[:, 512:1024], cch(1, 0, SCW))
            nc.gpsimd.tensor_mul(m2[:], gf[:, 1024:1536], cch(2, 0, SCW))
            nc.gpsimd.tensor_mul(m3[:], gf[:, 1536:2048], cch(3, 0, SCW))
            nc.vector.tensor_add(s01[:], m0[:], m1[:])
            nc.vector.tensor_add(s23[:], m2[:], m3[:])
            nc.vector.tensor_add(fc[:], s01[:], s23[:])
            nc.vector.tensor_mul(iu[:], gio[:, 0:512], gu[:])
            c_out = c_st[l][:, q0 : q0 + SCW]
            nc.vector.tensor_add(c_out, iu[:], fc[:])

            def tail():
                tct = mw.tile([128, SCW], F16, tag="tct1", name="tct1")
                nc.scalar.activation(tct[:], c_out, TANH)
                nc.vector.tensor_mul(h_st[l][:, q0 : q0 + SCW], gio[:, 512:1024], tct[:])

            return tail

        def _emit_forest():
            n1 = NL[1] // SCW  # 16 L1 units
            n2 = NL[2] // SCW  # 4 L2 units
            pend = []
            leaf_tiles = None

            def flush(limit):
                while len(pend) > limit:
                    pend.pop(0)()

            for sc in range(n1):
                flush(2)
                prev = leaf_tiles
                leaf_tiles = leaf_phase(sc)
                if sc >= 1:
                    h0, c0 = prev
                    pend.append(
                        internal_phase(
                            1,
                            sc - 1,
                            lambda k, a, b, h0=h0: h0[:, k, a:b],
                            lambda k, a, b, c0=c0: c0[:, k, a:b],
                        )
                    )
            flush(2)
            h0, c0 = leaf_tiles
            pend.append(
                internal_phase(
                    1,
                    n1 - 1,
                    lambda k, a, b, h0=h0: h0[:, k, a:b],
                    lambda k, a, b, c0=c0: c0[:, k, a:b],
                )
            )
            for u in range(n2):
                flush(2)
                pend.append(
                    internal_phase(
                        2,
                        u,
                        lambda k, a, b, u=u: h_st[1][
                            :, k * NL[2] + u * SCW + a : k * NL[2] + u * SCW + b
                        ],
                        lambda k, a, b, u=u: c_st[1][
                            :, k * NL[2] + u * SCW + a : k * NL[2] + u * SCW + b
                        ],
                    )
                )
            flush(0)

        for _rep in range(repeats):
            _emit_forest()

        # ---- outputs: h2|c2 fp16 -> [128, 4096] (levels 3..7 + root on host)
        nc.sync.dma_start(out_d.ap()[:, 0:2048], h_st[2][:])
        nc.sync.dma_start(out_d.ap()[:, 2048:4096], c_st[2][:])

    _split_excess_waits(nc)
    return nc


_PROGRAMS = {}


def _get_program(zero_bias: bool, repeats: int = 1):
    key = (bool(zero_bias), repeats)
    if key not in _PROGRAMS:
        _PROGRAMS[key] = _build_program(key[0], repeats=key[1])
    return _PROGRAMS[key]


def _orders():
    """Per-level child-major storage permutations (within-core natural index)."""
    ords = [None] * 8
    o = np.arange(2, dtype=np.int64)
    ords[7] = o
    for l in range(6, -1, -1):
        o = np.concatenate([4 * ords[l + 1] + k for k in range(4)])
        ords[l] = o
    return ords


def make_in_maps(x, Wx, Uiou, Uf, b):
    """Host-side shard/permute/transpose. Returns per-core input dicts."""
    x = np.asarray(x, dtype=np.float32)
    Wx = np.asarray(Wx, dtype=np.float32)
    Uiou = np.asarray(Uiou, dtype=np.float32)
    Uf = np.asarray(Uf, dtype=np.float32)
    b = np.asarray(b, dtype=np.float32)

    ords = _orders()
    wx_h = np.ascontiguousarray(Wx.astype(np.float16))
    uiou_h = np.ascontiguousarray(Uiou.astype(np.float16))
    uf_h = np.ascontiguousarray(Uf.astype(np.float16))
    bias_pg = np.ascontiguousarray(b.reshape(4, 128).T)  # [p, gate]

    in_maps = []
    for c in range(NCORES):
        tb, s = divmod(c, 2)
        xt = np.empty((128, NCOLS), np.float16)
        for l in range(8):
            nl = NL[l]
            xs = x[tb, OFFSETS[l] + s * nl : OFFSETS[l] + (s + 1) * nl, :]
            xt[:, LOFF[l] : LOFF[l] + nl] = xs[ords[l]].T
        in_maps.append(
            {"xt": xt, "wx": wx_h, "uiou": uiou_h, "uf": uf_h, "bias": bias_pg}
        )
    return in_maps


def finish_on_host(outs, x, Wx, Uiou, Uf, b):
    """Host combine: per-core levels 3..7 (682 tiny nodes) + the root level."""

    def sig(z):
        return 1.0 / (1.0 + np.exp(-z))

    x = np.asarray(x)
    Wx64 = np.asarray(Wx, np.float64)
    Uiou64 = np.asarray(Uiou, np.float64)
    Uf64 = np.asarray(Uf, np.float64)
    b64 = np.asarray(b, np.float64)
    ords = _orders()

    hc = np.empty((B, 4, H), np.float64)
    cc = np.empty((B, 4, H), np.float64)
    for core in range(NCORES):
        tb, s = divmod(core, 2)
        o = np.asarray(outs[core], np.float64)  # [128, 4096]
        h = o[:, 0:2048].T  # [2048 nodes, H] in L2 storage order
        c = o[:, 2048:4096].T
        for l in (3, 4, 5, 6, 7):
            nl = NL[l]
            hch = np.stack([h[k * nl : (k + 1) * nl] for k in range(4)], axis=1)
            cch = np.stack([c[k * nl : (k + 1) * nl] for k in range(4)], axis=1)
            xs = np.asarray(
                x[tb, OFFSETS[l] + s * nl + ords[l], :], np.float64
            )  # storage order
            g = xs @ Wx64 + b64
            xi, xf, xo, xu = np.split(g, 4, axis=1)
            hi, ho, hu = np.split(hch.sum(1) @ Uiou64, 3, axis=1)
            i = sig(xi + hi)
            og = sig(xo + ho)
            u = np.tanh(xu + hu)
            f = sig(xf[:, None, :] + hch @ Uf64)
            c = i * u + (f * cch).sum(1)
            h = og * np.tanh(c)
        hc[tb, 2 * s : 2 * s + 2] = h  # [2, H], storage order = natural
        cc[tb, 2 * s : 2 * s + 2] = c

    xr = np.asarray(x[:, OFFSETS[8], :], np.float64)  # [B, 128] root x
    g = xr @ Wx64 + b64
    xi, xf, xo, xu = np.split(g, 4, axis=1)
    hi, ho, hu = np.split(hc.sum(1) @ Uiou64, 3, axis=1)
    i = sig(xi + hi)
    o_ = sig(xo + ho)
    u = np.tanh(xu + hu)
    f = sig(xf[:, None, :] + hc @ Uf64)
    c = i * u + (f * cc).sum(1)
    h = o_ * np.tanh(c)
    return h.astype(np.float32), c.astype(np.float32)


def kernel(x, Wx, Uiou, Uf, b):
    x = np.asarray(x, dtype=np.float32)
    Wx = np.asarray(Wx, dtype=np.float32)
    Uiou = np.asarray(Uiou, dtype=np.float32)
    Uf = np.asarray(Uf, dtype=np.float32)
    b = np.asarray(b, dtype=np.float32)

    in_maps = make_in_maps(x, Wx, Uiou, Uf, b)
    nc = _get_program(zero_bias=not np.any(b))
    res = run_bass_kernel_spmd(nc, in_maps, list(range(NCORES)))
    outs = [res.results[c]["out"] for c in range(NCORES)]
    return finish_on_host(outs, x, Wx, Uiou, Uf, b)
